# revision 7
# baseline (speedup 1.0000x reference)
"""DeepseekMoE (8 experts, top-2, shared expert) on 8 Trainium2 NeuronCores.

Expert-parallel: core c computes expert c's SwiGLU FFN densely over all
T=1024 tokens, weighted by that expert's combine weight (0 for tokens that
did not select it), plus a 1/8 shard of the shared-expert FFN (intermediate
dim 2816 -> 8 x 352, zero-padded to 384). The router (fp32 matmul, softmax,
top-2) is computed on every core; each core extracts its own combine-weight
column. Per-core partial outputs [H, T] sum to the full MoE output, so the
host "unshard" is a reduce over the 8 cores plus a transpose back to
token-major.

All activations/weights on device are laid out feature-major ([feat, token])
so no on-device transposes of activations are needed. The three big GEMMs
per core run as float32r (single-pass, full PE rate at N=512, ~FP22 operand
precision). The router matmul runs in true fp32 (4-pass) because top-2
selection margins on the logits can be ~6e-5.
"""

import os
import numpy as np
from contextlib import ExitStack

# ---- problem shape (hardcoded per contract) ----
E = 8              # routed experts == n_cores
H = 2048           # hidden
I = 1408           # moe intermediate
IS = 2816          # shared intermediate total
B, S = 2, 512
T = B * S          # 1024 tokens
P = 128            # partitions
KT = H // P        # 16 contraction tiles over H
IT = I // P        # 11 I-tiles per expert
HT = H // P        # 16 output H-tiles
SS = IS // E       # 352 shared shard
SSP = 384          # shared shard padded to 3*128
ST = SSP // P      # 3
TN = 512           # token free-dim tile
NT = T // TN       # 2
N_CORES = 8

_CACHE = {}

LAST_EXEC_TIME_NS = None
LAST_TRACE_PATH = None


def _install_ntff_shim():
    """Register the axon NTFF profile hook (the agent image's antenv lacks it)
    and keep profile artifacts local. Only used when MOE_KERNEL_TRACE=1."""
    import sys, types
    import antenv

    if 'antenv.axon_hooks' not in sys.modules:
        mod = types.ModuleType('antenv.axon_hooks')
        _HOOK = [None]
        mod.set_axon_ntff_profile_hook = lambda h: _HOOK.__setitem__(0, h)
        mod.get_axon_ntff_profile_hook = lambda: _HOOK[0]
        sys.modules['antenv.axon_hooks'] = mod
        antenv.axon_hooks = mod
    from antenv.axon_hooks import set_axon_ntff_profile_hook as _set
    from trn_agent_boot.trn_boot import _ntff_profile_via_ctypes
    _set(_ntff_profile_via_ctypes('/opt/axon/libaxon_pjrt.so'))

    import concourse.bass_utils as bu
    bu.upload_artifacts = lambda tmpdir: tmpdir


def build_nc():
    """Emit the per-core SPMD program. Returns the compiled Bacc."""
    import concourse.bacc as bacc
    import concourse.tile as tile
    from concourse import mybir
    from concourse.masks import make_identity

    f32 = mybir.dt.float32
    f32r = mybir.dt.float32r
    AF = mybir.ActivationFunctionType
    ALU = mybir.AluOpType
    AX = mybir.AxisListType

    nc = bacc.Bacc("TRN2", target_bir_lowering=False, debug=False)

    # ---- DRAM I/O ----
    xt_d = nc.dram_tensor("xt", [P, KT, T], f32r, kind="ExternalInput")
    rwt_d = nc.dram_tensor("rwt", [P, KT, E], f32, kind="ExternalInput")
    sel_d = nc.dram_tensor("sel", [P, E], f32, kind="ExternalInput")
    wgu_d = nc.dram_tensor("wgu", [IT, P, KT, 2, P], f32r, kind="ExternalInput")
    swgu_d = nc.dram_tensor("swgu", [ST, P, KT, 2, P], f32r, kind="ExternalInput")
    wd_d = nc.dram_tensor("wd", [HT, P, IT + ST, P], f32r, kind="ExternalInput")
    out_d = nc.dram_tensor("out", [H, T], f32, kind="ExternalOutput")
    logits_d = nc.dram_tensor("logits", [E, T], f32, kind="ExternalOutput")

    with tile.TileContext(nc) as tc, ExitStack() as ctx:
        # ---- persistent SBUF ----
        persist = ctx.enter_context(tc.tile_pool(name="persist", bufs=1))
        xt = persist.tile([P, KT, T], f32r)          # 64KB/part
        rwt = persist.tile([P, KT, E], f32)
        sel = persist.tile([P, E], f32)
        ident = persist.tile([P, P], f32)
        ones = persist.tile([1, P], f32)
        h_sb = persist.tile([P, IT, T], f32r)        # 44KB/part
        sh_sb = persist.tile([P, ST, T], f32r)       # 12KB/part
        cw_bc = persist.tile([P, T], f32)           # 4KB/part
        cwrow = persist.tile([1, T], f32)
        lg_sb = persist.tile([E, T], f32)           # router logits [8, 1024]
        ltm = persist.tile([P, E, E], f32)          # token-major logits per T-tile of 128
        cwcol = persist.tile([P, E, 1], f32)        # combine weight col per T-tile

        make_identity(nc, ident)
        nc.vector.memset(ones[:], 1.0)

        # xt in 4 chunks so the router can start early
        for q in range(4):
            nc.sync.dma_start(xt[:, 4 * q:4 * q + 4, :], xt_d.ap()[:, 4 * q:4 * q + 4, :])
        nc.sync.dma_start(rwt[:], rwt_d.ap())
        nc.sync.dma_start(sel[:], sel_d.ap())

        # ---- phase R: router (true fp32 matmuls) ----
        ctx_r = ctx.enter_context(ExitStack())
        ps_r = ctx_r.enter_context(tc.tile_pool(name="ps_r", bufs=1, space="PSUM"))
        ps_s = ctx_r.enter_context(tc.tile_pool(name="ps_s", bufs=2, space="PSUM"))

        lg_ps = ps_r.tile([E, T], f32)   # 2 banks
        for t in range(NT):
            for k in range(KT):
                nc.tensor.matmul(
                    lg_ps[:, t * TN:(t + 1) * TN],
                    lhsT=rwt[:, k, :], rhs=xt[:, k, t * TN:(t + 1) * TN].bitcast(f32),
                    start=(k == 0), stop=(k == KT - 1))
        nc.scalar.copy(lg_sb[:], lg_ps[:])
        nc.sync.dma_start(logits_d.ap(), lg_sb[:])

        # transpose logits to token-major [128, 8] per 128-token tile
        for tt in range(E):
            tp = ps_s.tile([P, E], f32, tag="tp")
            nc.tensor.transpose(tp[:], lg_sb[:, tt * P:(tt + 1) * P], ident[0:E, 0:E])
            nc.scalar.copy(ltm[:, tt, :], tp[:])

        # ---- top-2 softmax combine weight for THIS core's expert ----
        small = ctx_r.enter_context(tc.tile_pool(name="small", bufs=2))
        for tt in range(E):
            lrow = ltm[:, tt, :]                       # [128, 8]
            srt = small.tile([P, E], f32, tag="srt")   # sorted desc
            nc.vector.max(srt[:], lrow)
            e_all = small.tile([P, E], f32, tag="eall")
            s_sum = small.tile([P, 1], f32, tag="ssum")
            nc.scalar.activation(e_all[:], lrow, AF.Exp, accum_out=s_sum[:])
            e12 = small.tile([P, 2], f32, tag="e12")
            nc.scalar.activation(e12[:], srt[:, 0:2], AF.Exp)
            den = small.tile([P, 1], f32, tag="den")
            # den = 1e-6*S + e1
            nc.vector.scalar_tensor_tensor(
                den[:], in0=s_sum[:], scalar=1e-6, in1=e12[:, 0:1],
                op0=ALU.mult, op1=ALU.add)
            nc.vector.tensor_add(den[:], den[:], e12[:, 1:2])
            rden = small.tile([P, 1], f32, tag="rden")
            nc.vector.reciprocal(rden[:], den[:])
            # cw8 = (l >= sorted2nd) * e_all
            cw8 = small.tile([P, E], f32, tag="cw8")
            nc.vector.scalar_tensor_tensor(
                cw8[:], in0=lrow, scalar=srt[:, 1:2], in1=e_all[:],
                op0=ALU.is_ge, op1=ALU.mult)
            # cw8 *= rden ; cw8 *= onehot(sel)
            nc.vector.scalar_tensor_tensor(
                cw8[:], in0=cw8[:], scalar=rden[:], in1=sel[:],
                op0=ALU.mult, op1=ALU.mult)
            nc.vector.tensor_reduce(cwcol[:, tt, :], cw8[:], axis=AX.X, op=ALU.add)

        # transpose cw columns into a row [1, T], then broadcast to 128 partitions
        for tt in range(E):
            tp = ps_s.tile([1, P], f32, tag="cwT")
            nc.tensor.transpose(tp[:], cwcol[:, tt, :], ident[:])
            nc.scalar.copy(cwrow[0:1, tt * P:(tt + 1) * P], tp[:])
        for t in range(NT):
            bc = ps_s.tile([P, TN], f32, tag="bc")
            nc.tensor.matmul(bc[:], lhsT=ones[:], rhs=cwrow[0:1, t * TN:(t + 1) * TN],
                             start=True, stop=True)
            nc.scalar.copy(cw_bc[:, t * TN:(t + 1) * TN], bc[:])

        ctx_r.close()  # release router PSUM/SBUF pools for the gate phase

        # ---- phase G: routed expert gate/up -> h ; shared gate/up -> sh ----
        ctx_g = ctx.enter_context(ExitStack())
        wgu_pool = ctx_g.enter_context(tc.tile_pool(name="wgu", bufs=2))
        ps_g = ctx_g.enter_context(tc.tile_pool(name="ps_g", bufs=8, space="PSUM"))
        ev = ctx_g.enter_context(tc.tile_pool(name="ev", bufs=3))

        def gate_up(i, w_sb, dst, shared):
            """One I-tile of gate+up + SwiGLU eviction into dst[:, i, :]."""
            for t in range(NT):
                tsl = slice(t * TN, (t + 1) * TN)
                g_ps = ps_g.tile([P, TN], f32, tag="gu_ps")
                u_ps = ps_g.tile([P, TN], f32, tag="gu_ps")
                for k in range(KT):
                    nc.tensor.matmul(
                        g_ps[:], lhsT=w_sb[:, k, 0, :],
                        rhs=xt[:, k, tsl],
                        start=(k == 0), stop=(k == KT - 1))
                for k in range(KT):
                    nc.tensor.matmul(
                        u_ps[:], lhsT=w_sb[:, k, 1, :],
                        rhs=xt[:, k, tsl],
                        start=(k == 0), stop=(k == KT - 1))
                sg = ev.tile([P, TN], f32, tag="sg")
                nc.scalar.activation(sg[:], g_ps[:], AF.Sigmoid)
                nc.vector.tensor_mul(sg[:], sg[:], g_ps[:])   # silu(g) = g*sigmoid(g)
                if shared:
                    # sh = silu(g) * u
                    nc.vector.tensor_mul(dst[:, i, tsl], sg[:], u_ps[:])
                else:
                    # h = silu(g) * (u * cw)
                    usc = ev.tile([P, TN], f32, tag="usc")
                    nc.vector.tensor_mul(usc[:], u_ps[:], cw_bc[:, tsl])
                    nc.vector.tensor_mul(dst[:, i, tsl], sg[:], usc[:])

        for i in range(IT):
            w_sb = wgu_pool.tile([P, KT, 2, P], f32r, tag="wgu")
            nc.sync.dma_start(w_sb[:], wgu_d.ap()[i])
            gate_up(i, w_sb, h_sb, shared=False)
        for s in range(ST):
            w_sb = wgu_pool.tile([P, KT, 2, P], f32r, tag="wgu")
            nc.sync.dma_start(w_sb[:], swgu_d.ap()[s])
            gate_up(s, w_sb, sh_sb, shared=True)

        ctx_g.close()  # release gate-phase pools

        # ---- phase D: down projection, routed + shared accumulated in PSUM ----
        wd_pool = ctx.enter_context(tc.tile_pool(name="wd", bufs=2))
        ps_d = ctx.enter_context(tc.tile_pool(name="ps_d", bufs=4, space="PSUM"))
        evd = ctx.enter_context(tc.tile_pool(name="evd", bufs=3))
        for h in range(HT):
            wd_sb = wd_pool.tile([P, IT + ST, P], f32r, tag="wd")
            nc.sync.dma_start(wd_sb[:], wd_d.ap()[h])
            for t in range(NT):
                tsl = slice(t * TN, (t + 1) * TN)
                o_ps = ps_d.tile([P, TN], f32, tag="o_ps")
                for j in range(IT):
                    nc.tensor.matmul(
                        o_ps[:], lhsT=wd_sb[:, j, :],
                        rhs=h_sb[:, j, tsl],
                        start=(j == 0), stop=False)
                for s in range(ST):
                    nc.tensor.matmul(
                        o_ps[:], lhsT=wd_sb[:, IT + s, :],
                        rhs=sh_sb[:, s, tsl],
                        start=False, stop=(s == ST - 1))
                o_sb = evd.tile([P, TN], f32, tag="o_sb")
                nc.scalar.copy(o_sb[:], o_ps[:])
                nc.sync.dma_start(out_d.ap()[h * P:(h + 1) * P, tsl], o_sb[:])

    nc.compile()
    return nc


def _prep_in_maps(hidden_states, router_w, w_gate, w_up, w_down,
                  sw_gate, sw_up, sw_down):
    f = np.float32
    x = np.ascontiguousarray(np.asarray(hidden_states, dtype=f)).reshape(T, H)
    # xt[p, k, t] = x[t, 128k+p]
    xt = np.ascontiguousarray(x.T.reshape(KT, P, T).transpose(1, 0, 2))
    rw = np.asarray(router_w, dtype=f)
    rwt = np.ascontiguousarray(rw.T.reshape(KT, P, E).transpose(1, 0, 2))

    w_gate = np.asarray(w_gate, dtype=f)
    w_up = np.asarray(w_up, dtype=f)
    w_down = np.asarray(w_down, dtype=f)
    sw_gate = np.asarray(sw_gate, dtype=f)
    sw_up = np.asarray(sw_up, dtype=f)
    sw_down = np.asarray(sw_down, dtype=f)

    in_maps = []
    for c in range(N_CORES):
        # wgu[i, p, k, {g,u}, m] = w[128i+m, 128k+p]
        def tile_iu(w):  # [I, H] -> [IT, P, KT, P]
            return w.reshape(IT, P, KT, P).transpose(0, 3, 2, 1)
        wgu = np.ascontiguousarray(
            np.stack([tile_iu(w_gate[c]), tile_iu(w_up[c])], axis=3))

        sg = np.zeros((SSP, H), f)
        su = np.zeros((SSP, H), f)
        sg[:SS] = sw_gate[c * SS:(c + 1) * SS]
        su[:SS] = sw_up[c * SS:(c + 1) * SS]

        def tile_su(w):  # [SSP, H] -> [ST, P, KT, P]
            return w.reshape(ST, P, KT, P).transpose(0, 3, 2, 1)
        swgu = np.ascontiguousarray(np.stack([tile_su(sg), tile_su(su)], axis=3))

        # wd[h, p, j, m]: j<IT routed w_down[c][128h+m, 128j+p]; j>=IT shared shard
        wdr = w_down[c].reshape(HT, P, IT, P).transpose(0, 3, 2, 1)
        sd = np.zeros((H, SSP), f)
        sd[:, :SS] = sw_down[:, c * SS:(c + 1) * SS]
        wds = sd.reshape(HT, P, ST, P).transpose(0, 3, 2, 1)
        wd = np.ascontiguousarray(np.concatenate([wdr, wds], axis=2))

        selv = np.zeros((P, E), f)
        selv[:, c] = 1.0

        in_maps.append({
            "xt": xt, "rwt": rwt, "sel": selv,
            "wgu": wgu, "swgu": swgu, "wd": wd,
        })
    return in_maps


def kernel(hidden_states, router_w, w_gate, w_up, w_down,
           sw_gate, sw_up, sw_down):
    global LAST_EXEC_TIME_NS, LAST_TRACE_PATH
    from concourse.bass_utils import run_bass_kernel_spmd

    trace = os.environ.get("MOE_KERNEL_TRACE", "0") == "1"
    if trace:
        _install_ntff_shim()

    if "nc" not in _CACHE:
        _CACHE["nc"] = build_nc()
    nc = _CACHE["nc"]

    in_maps = _prep_in_maps(hidden_states, router_w, w_gate, w_up, w_down,
                            sw_gate, sw_up, sw_down)
    res = run_bass_kernel_spmd(nc, in_maps, core_ids=list(range(N_CORES)),
                               trace=trace)
    if trace:
        LAST_EXEC_TIME_NS = res.exec_time_ns
        LAST_TRACE_PATH = (res.instructions_and_trace or (None, None))[1]

    acc = res.results[0]["out"].astype(np.float32)
    for c in range(1, N_CORES):
        acc += res.results[c]["out"]
    final = np.ascontiguousarray(acc.T).reshape(B, S, H)
    router_logits = np.ascontiguousarray(res.results[0]["logits"].T)
    return final, router_logits
